# revision 51
# baseline (speedup 1.0000x reference)
"""Trainium2 Bass kernel for a dense transformer block (B=2, T=2048, C=1024, H=16).

Sharding (v4, pipelined tensor-parallel attention + chunked ReduceScatter):
  core c -> batch b = c//4, head-group g = c%4 (heads 4g..4g+3).
  After the CHUNKED ReduceScatter, core g owns token rows
  {qc*512 + 128*g + [0,128) : qc in 0..3} of its batch (strided, 4 tiles).

Emission is software-pipelined per query chunk qc so all five engines
overlap: LN1 stats/x-tile DMAs run two chunks ahead; LN-apply+transpose and
proj(qc-1) drain as background units inside C(qc).pt0's k-tile loop; qkv of
qc+1 drains inside C(qc).pt1; the ReduceScatter chunks fly under later
attention; E(j) residual+LN2 lags its RS by two chunks.  The MLP runs fc1
in token-column splits (256/128/128) so the last RS chunk + E(3) hide under
fc1, then a token-major fc2 (no output transposes).

Attention processes only valid (causal) score columns: diagonal-band k-tiles
compute/exp/mask just cols >= 128*band, with a single static 128x128
staircase mask.

Everything is bf16 except PSUM accumulation, LN statistics and the final
residual path (f32).  LayerNorm affines are folded into the following matmul
weights on the host (exact).  Weights are pre-laid-out on the host so every
DMA moves >=2KB contiguous runs per partition.
"""

from contextlib import ExitStack

import ml_dtypes
import numpy as np

import concourse.bass as bass
import concourse.tile as tile
import concourse.bacc as bacc
import concourse.mybir as mybir
from concourse.bass_utils import run_bass_kernel_spmd
from concourse.masks import make_identity

F32 = mybir.dt.float32
BF16 = mybir.dt.bfloat16
ALU = mybir.AluOpType
ACTF = mybir.ActivationFunctionType

B, T, C = 2, 2048, 1024
H, DH = 16, 64
FF = 4096
EPS = 1e-5
NCORES = 8
ROWS = 512            # token rows owned per core (MLP phase)
HG = 4                # heads per core
HGF = HG * DH         # 256 head-group features
NCP = C // 128        # 8 feature partition-tiles of C
NFP = FF // 128       # 32 feature partition-tiles of FF
NQC = T // 512        # 4 query chunks
VSTRIDE = DH + 1      # v stored with a ones column per head


def r(ap, pat, **kw):
    return ap.rearrange(pat, **kw)


def build_program():
    nc = bacc.Bacc("TRN2", target_bir_lowering=False, debug=False,
                   num_devices=NCORES)

    def din(name, shape, dtype=F32):
        return nc.dram_tensor(name, list(shape), dtype, kind="ExternalInput")

    xb = din("xb", (T, C), BF16)
    xo_d = din("xo", (ROWS, C))
    cmask = din("cmask", (128, 128), BF16)
    w_qkv = din("w_qkv", (128, NCP, 3 * HGF), BF16)   # [p, kt, q|k|v feats]
    w_pr = din("w_pr", (2, 128, C), BF16)             # proj rows, 2 p-tiles
    w_fc = din("w_fc", (128, NFP, NCP, 128), BF16)    # [p, m, kt, c]
    w_fc2 = din("w_fc2", (128, 4, NFP, 256), BF16)    # [p, ccq, k2, c]
    b_qk_col = din("b_qk_col", (128, 4))              # q0 q1 k0 k1 bias cols
    b_v_bc = din("b_v_bc", (128, HGF))
    b_proj_bc = din("b_proj_bc", (128, C), BF16)
    b_fc_col = din("b_fc_col", (128, 32))
    b_fc2_bc = din("b_fc2_bc", (128, C), BF16)
    out = nc.dram_tensor("out", [ROWS, C], F32, kind="ExternalOutput")

    with tile.TileContext(nc) as tc, ExitStack() as ctx:
        # ---- constants (whole-program lifetime) ----
        cpool = ctx.enter_context(tc.tile_pool(name="const", bufs=1))
        ident = cpool.tile([128, 128], F32, tag="ident")
        make_identity(nc, ident[:])
        identb = cpool.tile([128, 128], BF16, tag="identb")
        nc.vector.tensor_copy(identb[:], ident[:])
        bqk = cpool.tile([128, 4], F32, tag="bqk")
        bvbc = cpool.tile([128, HGF], F32, tag="bvbc")
        bprbc = cpool.tile([128, C], BF16, tag="bprbc")
        bfc = cpool.tile([128, 32], F32, tag="bfc")
        bfc2bc = cpool.tile([128, C], BF16, tag="bfc2bc")
        mtile = cpool.tile([128, 128], BF16, tag="mtile")
        epsc = cpool.tile([128, 1], F32, tag="epsc")
        nc.gpsimd.memset(epsc[:], EPS)
        # dummy Sqrt pulls its activation-table load to t=0 (otherwise it
        # gates the first LayerNorm ~6us into the run)
        warm = cpool.tile([128, 1], F32, tag="warm")
        nc.scalar.activation(warm[:], epsc[:], ACTF.Sqrt, bias=epsc[:],
                             scale=1.0)

        def load_consts():
            # deferred so these DMAs queue behind the critical first x tiles
            nc.sync.dma_start(bqk[:], b_qk_col.ap())
            nc.sync.dma_start(bvbc[:], b_v_bc.ap())
            nc.sync.dma_start(bprbc[:], b_proj_bc.ap())
            nc.sync.dma_start(bfc[:], b_fc_col.ap())
            nc.sync.dma_start(bfc2bc[:], b_fc2_bc.ap())
            nc.sync.dma_start(mtile[:], cmask.ap())

        def newton_rsqrt(spool, y, ve, iters):
            """y = rsqrt(ve) via Newton from seed y1 = 1.5 - 0.5*ve.
            All-DVE: keeps Sqrt (and its activation-table load) out of the
            ACT exp stream.  Valid for ve in ~(0, 2.5); LN variances here
            are ~1."""
            n = y.shape[-1]
            nc.vector.tensor_scalar(y, ve, -0.5, 1.5, op0=ALU.mult,
                                    op1=ALU.add)
            for _ in range(iters):
                t = spool.tile([128, n], F32, tag="nt")
                nc.vector.tensor_tensor(t[:], y, y, op=ALU.mult)
                nc.vector.tensor_tensor(t[:], t[:], ve, op=ALU.mult)
                nc.vector.tensor_scalar(t[:], t[:], -0.5, 1.5, op0=ALU.mult,
                                        op1=ALU.add)
                nc.vector.tensor_tensor(y, y, t[:], op=ALU.mult)

        def layernorm_apply(spool, xt, xn_out, iters=4):
            """xn_out = (xt - mean) * rsqrt(var + eps), rowwise over 1024."""
            st = spool.tile([128, 12], F32, tag="st")
            nc.vector.bn_stats(st[:, 0:6], xt[:, 0:512])
            nc.vector.bn_stats(st[:, 6:12], xt[:, 512:1024])
            ag = spool.tile([128, 2], F32, tag="ag")
            nc.vector.bn_aggr(ag[:], r(st, "p (c s) -> p c s", s=6))
            sd = spool.tile([128, 1], F32, tag="sd")
            nc.scalar.activation(sd[:], ag[:, 1:2], ACTF.Sqrt, bias=epsc[:],
                                 scale=1.0)
            rc = spool.tile([128, 1], F32, tag="rc")
            nc.vector.reciprocal(rc[:], sd[:])
            # xn = (x - mu) * r on DVE (bf16 in/out hits the 4x fast path)
            nc.vector.tensor_scalar(xn_out, xt, ag[:, 0:1], rc[:],
                                    op0=ALU.subtract, op1=ALU.mult)

        # DRAM bounce buffers for the chunked collective
        drpool = ctx.enter_context(tc.tile_pool(name="dram", bufs=1,
                                                space="DRAM"))
        pp_d = drpool.tile([T, C], BF16, tag="pp_d", name="pp_d")
        rs_d = drpool.tile([ROWS, C], BF16, tag="rs_d", name="rs_d")

        # ---- MLP-persistent state + shared transpose PSUM (outer scope,
        # so attention pools can close LIFO around them) ----
        mpers = ctx.enter_context(tc.tile_pool(name="mpers", bufs=1))
        x2 = [mpers.tile([128, C], F32, tag=f"x2{j}", name=f"x2{j}")
              for j in range(4)]
        xn2T = mpers.tile([128, NCP, ROWS], BF16, tag="xn2T", name="xn2T")
        hgT = mpers.tile([128, NFP, ROWS], BF16, tag="hgT", name="hgT")
        epool = ctx.enter_context(tc.tile_pool(name="phE", bufs=3))
        espool = ctx.enter_context(tc.tile_pool(name="phEs", bufs=4))
        wf0pool = ctx.enter_context(tc.tile_pool(name="wf0", bufs=1))
        wf0 = wf0pool.tile([128, 4, NCP, 128], BF16, tag="wf0", name="wf0")
        atps = ctx.enter_context(
            tc.tile_pool(name="atps", bufs=1, space="PSUM"))

        def phase_e(j, dma=None):
            dma = dma or nc.sync
            rs_sb = epool.tile([128, C], BF16, tag="rs")
            dma.dma_start(rs_sb[:], rs_d[j * 128:(j + 1) * 128, :])
            xot = epool.tile([128, C], F32, tag="xot")
            dma.dma_start(xot[:], xo_d.ap()[j * 128:(j + 1) * 128, :])
            xa = epool.tile([128, C], BF16, tag="xa")
            nc.vector.tensor_tensor(xa[:], rs_sb[:], bprbc[:], op=ALU.add)
            nc.vector.tensor_tensor(x2[j][:], xa[:], xot[:], op=ALU.add)
            xn2 = epool.tile([128, C], BF16, tag="xn2")
            layernorm_apply(espool, x2[j][:], xn2[:])
            for half in range(2):
                tp = atps.tile([128, 512], BF16, tag="tp",
                               name=f"etp_{j}_{half}")
                for pq in range(4):
                    pt = 4 * half + pq
                    nc.tensor.transpose(
                        tp[:, pq * 128:(pq + 1) * 128],
                        xn2[:, pt * 128:(pt + 1) * 128],
                        identb[:])
                nc.vector.tensor_copy(
                    xn2T[:, 4 * half:4 * half + 4, j * 128:(j + 1) * 128],
                    r(tp[:], "p (k c) -> p k c", c=128))

        # =================== attention super-phase =====================
        with ExitStack() as actx:
            kvp = actx.enter_context(tc.tile_pool(name="kv", bufs=1))
            kTb = [kvp.tile([128, T], BF16, tag=f"kT{i}", name=f"kT{i}")
                   for i in range(2)]
            vb = [kvp.tile([128, 2, HG * VSTRIDE], BF16, tag=f"v{i}",
                           name=f"v{i}") for i in range(T // 256)]
            wq = kvp.tile([128, NCP, 3 * HGF], BF16, tag="wq", name="wq")
            wp = [kvp.tile([128, C], BF16, tag=f"wp{i}", name=f"wp{i}")
                  for i in range(2)]
            xnTp = actx.enter_context(tc.tile_pool(name="xnT", bufs=1))
            qTp = actx.enter_context(tc.tile_pool(name="qT", bufs=1))
            yTp = actx.enter_context(tc.tile_pool(name="yT", bufs=1))
            apool = actx.enter_context(tc.tile_pool(name="phA", bufs=4))
            aspool = actx.enter_context(tc.tile_pool(name="phAs", bufs=6))
            smpool = actx.enter_context(tc.tile_pool(name="sm", bufs=3))
            atpool = actx.enter_context(tc.tile_pool(name="att", bufs=6))
            depool = actx.enter_context(tc.tile_pool(name="phDe", bufs=3))
            # PSUM: atps(outer) 0.5 + mm 2 + sc 3x1 + av 2 = 7.5 banks
            mmps = actx.enter_context(
                tc.tile_pool(name="mmps", bufs=2, space="PSUM"))
            scps = actx.enter_context(
                tc.tile_pool(name="scps", bufs=3, space="PSUM"))
            avps = actx.enter_context(
                tc.tile_pool(name="avps", bufs=1, space="PSUM"))

            def dma_x(qc):
                # prefetch the chunk's x tiles ~20us ahead so bn_stats never
                # holds the DVE queue waiting on a DMA
                tiles = []
                for tloc in range(4):
                    tt = 4 * qc + tloc
                    xt = apool.tile([128, C], BF16, tag="x", bufs=8,
                                    name=f"x_{tt}")
                    nc.sync.dma_start(xt[:],
                                      xb.ap()[tt * 128:(tt + 1) * 128, :])
                    tiles.append(xt)
                return tiles

            def stats_alloc():
                agq = aspool.tile([128, 4, 2], F32, tag="agq")
                rcq = aspool.tile([128, 4], F32, tag="rcq")
                sdq = aspool.tile([128, 4], F32, tag="sdq")
                return agq, rcq, sdq

            def stats_one(tloc, xt, agq, rcq, sdq, per_tile=False):
                st = aspool.tile([128, 12], F32, tag="st")
                nc.vector.bn_stats(st[:, 0:6], xt[:, 0:512])
                nc.vector.bn_stats(st[:, 6:12], xt[:, 512:1024])
                nc.vector.bn_aggr(agq[:, tloc, :],
                                  r(st, "p (c s) -> p c s", s=6))
                if per_tile:
                    nc.scalar.activation(sdq[:, tloc:tloc + 1],
                                         agq[:, tloc, 1:2], ACTF.Sqrt,
                                         bias=epsc[:], scale=1.0)
                    nc.vector.reciprocal(rcq[:, tloc:tloc + 1],
                                         sdq[:, tloc:tloc + 1])

            def phase_a_stats(qc, tiles):
                # LN stats (DVE) -- emitted ahead so the latency hides under
                # the previous exp stream
                agq, rcq, sdq = stats_alloc()
                for tloc in range(4):
                    stats_one(tloc, tiles[tloc], agq, rcq, sdq)
                nc.scalar.activation(sdq[:], agq[:, :, 1], ACTF.Sqrt,
                                     bias=epsc[:], scale=1.0)
                nc.vector.reciprocal(rcq[:], sdq[:])
                return tiles, agq, rcq

            def a_apply_units(qc, stats):
                """Per-tile LN-apply + transpose closures (bg units)."""
                tiles, agq, rcq = stats
                xnTq = xnTp.tile([128, NCP, 512], BF16, tag="xnT", bufs=2,
                                 name=f"xnT{qc}")

                def unit(tloc, xt):
                    xn = apool.tile([128, C], BF16, tag="xn")
                    nc.vector.tensor_scalar(xn[:], xt[:],
                                            agq[:, tloc, 0:1],
                                            rcq[:, tloc:tloc + 1],
                                            op0=ALU.subtract, op1=ALU.mult)
                    for half in range(2):
                        tp = atps.tile([128, 512], BF16, tag="tp",
                                       name=f"tp_{qc}_{tloc}_{half}")
                        for pq in range(4):
                            pt = 4 * half + pq
                            nc.tensor.transpose(
                                tp[:, pq * 128:(pq + 1) * 128],
                                xn[:, pt * 128:(pt + 1) * 128],
                                identb[:])
                        nc.vector.tensor_copy(
                            xnTq[:, 4 * half:4 * half + 4,
                                 tloc * 128:(tloc + 1) * 128],
                            r(tp[:], "p (k c) -> p k c", c=128))

                units = [lambda tloc=tloc, xt=xt: unit(tloc, xt)
                         for tloc, xt in enumerate(tiles)]
                return xnTq, units

            def b_units(qc, xnTq):
                """qkv matmul-group closures (bg units): 8 qk half-groups
                + 4 v groups."""
                qTb = [qTp.tile([128, 512], BF16, tag=f"qT{i}", bufs=2,
                                name=f"qT{i}_{qc}") for i in range(2)]

                def v_unit(tloc):
                    tt = 4 * qc + tloc
                    ps = mmps.tile([128, 512], F32, tag="mm",
                                   name=f"v_{tt}")
                    for kt in range(NCP):
                        nc.tensor.matmul(
                            ps[:, 0:HGF],
                            xnTq[:, kt, tloc * 128:(tloc + 1) * 128],
                            wq[:, kt, 2 * HGF:3 * HGF],
                            start=(kt == 0), stop=(kt == NCP - 1))
                    dst = r(vb[tt // 2][:, tt % 2, :], "p (h m) -> p h m",
                            m=VSTRIDE)[:, :, 0:DH]
                    nc.vector.tensor_tensor(
                        dst, r(ps[:, 0:HGF], "p (h m) -> p h m", m=DH),
                        r(bvbc[:], "p (h m) -> p h m", m=DH), op=ALU.add)

                units = [lambda tloc=tloc: v_unit(tloc) for tloc in range(4)]
                for m in range(4):
                    # m: 0,1 -> q head-pairs; 2,3 -> k head-pairs
                    holder = []

                    def qk_first(m=m, holder=holder):
                        ps = mmps.tile([128, 512], F32, tag="mm",
                                       name=f"qk_{qc}_{m}")
                        holder.append(ps)
                        for kt in range(4):
                            nc.tensor.matmul(
                                ps[:], wq[:, kt, m * 128:(m + 1) * 128],
                                xnTq[:, kt, :],
                                start=(kt == 0), stop=False)

                    def qk_second(m=m, holder=holder):
                        ps = holder[0]
                        for kt in range(4, NCP):
                            nc.tensor.matmul(
                                ps[:], wq[:, kt, m * 128:(m + 1) * 128],
                                xnTq[:, kt, :],
                                start=False, stop=(kt == NCP - 1))
                        if m < 2:
                            dst = qTb[m][:]
                        else:
                            dst = kTb[m - 2][:, qc * 512:(qc + 1) * 512]
                        nc.vector.tensor_scalar(dst, ps[:], bqk[:, m:m + 1],
                                                None, op0=ALU.add)

                    units += [qk_first, qk_second]
                return qTb, units

            def attn_head_pair(qc, pt, qTb, yTq, bg=None):
                """Both heads of pair pt, kt-pipelined: av lags sc by one
                k-tile so PE never waits on the exp.  One background unit
                (a closure emitting ~0.5-1us of boundary work) is drained
                per k-tile to fill PE slack under the ACT-paced stream."""
                nkt = 4 * (qc + 1)
                avs = [avps.tile([128, 512], F32, tag=f"av{sub}",
                                 name=f"av_{qc}_{pt}_{sub}")
                       for sub in range(2)]

                def emit_av(entries):
                    for sub, (kt, c0, et) in enumerate(entries):
                        h = 2 * pt + sub
                        nc.tensor.matmul(
                            avs[sub][0:VSTRIDE, c0:512],
                            vb[kt // 2][:, kt % 2,
                                        h * VSTRIDE:(h + 1) * VSTRIDE],
                            et[:, c0:512],
                            start=(kt == 0), stop=(kt == nkt - 1),
                            skip_group_check=True)

                prev = None
                for kt in range(nkt):
                    band = kt - 4 * qc
                    c0 = 128 * band if band > 0 else 0
                    cur = []
                    for sub in range(2):
                        hb = 64 * sub
                        sc = scps.tile([128, 512], F32, tag="sc")
                        nc.tensor.matmul(
                            sc[:, c0:512],
                            kTb[pt][hb:hb + 64, kt * 128:(kt + 1) * 128],
                            qTb[pt][hb:hb + 64, c0:512],
                            start=True, stop=True)
                        et = atpool.tile([128, 512], BF16, tag="e", bufs=6)
                        nc.scalar.activation(et[:, c0:512], sc[:, c0:512],
                                             ACTF.Exp, scale=0.125)
                        if band >= 0:
                            # staircase-mask the diagonal stripe (on Pool)
                            nc.gpsimd.tensor_tensor(
                                et[:, c0:c0 + 128], et[:, c0:c0 + 128],
                                mtile[:], op=ALU.mult)
                        cur.append((kt, c0, et))
                    if prev is not None:
                        emit_av(prev)
                    if bg:
                        bg.pop(0)()
                    prev = cur
                emit_av(prev)
                for sub in range(2):
                    hb = 64 * sub
                    rr = smpool.tile([1, 512], F32, tag="rr")
                    nc.vector.reciprocal(rr[:], avs[sub][DH:DH + 1, :])
                    bc = smpool.tile([64, 512], F32, tag="bc")
                    nc.gpsimd.partition_broadcast(bc[:], rr[:])
                    nc.vector.tensor_tensor(
                        yTq[pt][hb:hb + 64, :], avs[sub][0:DH, :], bc[:],
                        op=ALU.mult)

            def proj_unit(qc, tloc, yTq):
                tt = 4 * qc + tloc
                pe = depool.tile([128, C], BF16, tag="pe")
                pj = [mmps.tile([128, 512], F32, tag="mm",
                                name=f"pj_{tt}_{cc}") for cc in range(2)]
                for i in range(2):
                    for cc in range(2):
                        nc.tensor.matmul(
                            pj[cc][:],
                            yTq[i][:, tloc * 128:(tloc + 1) * 128],
                            wp[i][:, cc * 512:(cc + 1) * 512],
                            start=(i == 0), stop=(i == 1))
                nc.vector.tensor_copy(pe[:, 0:512], pj[0][:])
                nc.scalar.copy(pe[:, 512:1024], pj[1][:])
                nc.sync.dma_start(pp_d[tt * 128:(tt + 1) * 128, :], pe[:])

            def emit_rs(qc):
                nc.gpsimd.collective_compute(
                    "ReduceScatter", ALU.add,
                    replica_groups=[[0, 1, 2, 3], [4, 5, 6, 7]],
                    ins=[pp_d[qc * 512:(qc + 1) * 512, :]],
                    outs=[rs_d[qc * 128:(qc + 1) * 128, :]])

            def phase_d(qc, yTq):
                for tloc in range(4):
                    proj_unit(qc, tloc, yTq)
                emit_rs(qc)

            # ---- pipelined emission: stats of chunk qc+1 go ahead of
            # C(qc).pt0; LN-apply + proj(qc-1) drain as background units
            # inside pt0's kt loop; qkv(qc+1) units inside pt1's loop.
            # The exp stream never breaks and PE slack is backfilled. ----
            xts = {0: dma_x(0)}
            nc.sync.dma_start(wq[:], w_qkv.ap())
            for i in range(2):
                nc.sync.dma_start(wp[i][:], w_pr.ap()[i])
            xts[1] = dma_x(1)
            load_consts()
            for i in range(T // 256):
                nc.gpsimd.memset(
                    r(vb[i], "p s (h m) -> p s h m",
                      m=VSTRIDE)[:, :, :, DH:DH + 1], 1.0)
            # prologue: per-tile stats -> apply -> v interleave for min
            # latency (v of tile t only needs tile t transposed)
            agq0, rcq0, sdq0 = stats_alloc()
            xnTq, a_units = a_apply_units(0, (xts[0], agq0, rcq0))
            qTb, bu = b_units(0, xnTq)
            for tloc in range(4):
                stats_one(tloc, xts[0][tloc], agq0, rcq0, sdq0,
                          per_tile=True)
                a_units[tloc]()
                bu[tloc]()      # v unit for this tile
            for u in bu[4:]:
                u()
            yTq_prev = None
            for qc in range(NQC):
                yTq = [yTp.tile([128, 512], BF16, tag=f"yT{i}", bufs=2,
                                name=f"yT{i}_{qc}") for i in range(2)]
                if qc + 2 < NQC:
                    xts[qc + 2] = dma_x(qc + 2)
                bg0 = []
                if yTq_prev is not None:
                    bg0 += [lambda tloc=tloc, y=yTq_prev:
                            proj_unit(qc - 1, tloc, y) for tloc in range(4)]
                if qc + 1 < NQC:
                    stats_n = phase_a_stats(qc + 1, xts[qc + 1])
                    xnTq_n, a_un = a_apply_units(qc + 1, stats_n)
                    bg0 += a_un
                attn_head_pair(qc, 0, qTb, yTq, bg=bg0)
                for u in bg0:
                    u()
                bg0.clear()
                if qc + 1 < NQC:
                    qTb_n, bg1 = b_units(qc + 1, xnTq_n)
                else:
                    qTb_n, bg1 = None, []
                attn_head_pair(qc, 1, qTb, yTq, bg=bg1)
                for u in bg1:
                    u()
                bg1.clear()
                # RS(qc-1) emitted here so its SEQ-held wait on the pp
                # stores (long satisfied) never blocks Pool-queue masks
                if yTq_prev is not None:
                    emit_rs(qc - 1)
                if qc == 3:
                    phase_d(3, yTq)
                # E lags TWO chunks so its rs DMA never waits (a waiting
                # DMA holds the SP queue and stalls every later DMA)
                if qc >= 2:
                    phase_e(qc - 2)
                if qc == 3:
                    phase_e(2)
                if qc == 2:
                    nc.sync.dma_start(wf0[:], w_fc.ap()[:, 0:4])
                qTb = qTb_n
                yTq_prev = yTq

        # =================== MLP super-phase ===========================
        with ExitStack() as mctx:
            fpool = mctx.enter_context(tc.tile_pool(name="phF", bufs=1))
            f2pool = mctx.enter_context(tc.tile_pool(name="phF2", bufs=2))
            opool = mctx.enter_context(tc.tile_pool(name="phO", bufs=3))
            fps = mctx.enter_context(
                tc.tile_pool(name="fps", bufs=2, space="PSUM"))
            f2ps = mctx.enter_context(
                tc.tile_pool(name="f2ps", bufs=2, space="PSUM"))

            # fc1 weights resident (mg=0 was prefetched during attention);
            # wf2 quarters prefetch
            wf = [wf0] + [fpool.tile([128, 4, NCP, 128], BF16, tag=f"wf{mg}",
                                     name=f"wf{mg}") for mg in range(1, 8)]
            for mg in range(1, 8):
                nc.sync.dma_start(wf[mg][:], w_fc.ap()[:, 4 * mg:4 * mg + 4])
            wf2 = [f2pool.tile([128, NFP, 256], BF16, tag="wf2",
                               name=f"wf2_{q}") for q in range(4)]
            nc.sync.dma_start(wf2[0][:], w_fc2.ap()[:, 0])
            nc.sync.dma_start(wf2[1][:], w_fc2.ap()[:, 1])

            def fc1_cols(lo, hi):
                for mg in range(8):
                    for mloc in range(4):
                        m = 4 * mg + mloc
                        ps = fps.tile([128, hi - lo], F32,
                                      tag=f"fc{hi - lo}", name=f"fc_{m}_{lo}")
                        for kt in range(NCP):
                            nc.tensor.matmul(
                                ps[:], wf[mg][:, mloc, kt, :],
                                xn2T[:, kt, lo:hi],
                                start=(kt == 0), stop=(kt == NCP - 1))
                        nc.scalar.activation(hgT[:, m, lo:hi], ps[:],
                                             ACTF.Gelu, bias=bfc[:, m:m + 1],
                                             scale=1.0)

            # chunks 0,1 first (E(0),E(1) long done); cols 256:384 follow
            # once E(2)'s chain lands; the last RS + E(3) hide underneath
            fc1_cols(0, 256)
            fc1_cols(256, 384)
            phase_e(3)
            fc1_cols(384, 512)

            # fc2 token-major in c-quarters, out stripes DMA'd as they finish
            for ccq in range(4):
                if ccq >= 2:
                    nc.sync.dma_start(wf2[ccq][:], w_fc2.ap()[:, ccq])
                for j in range(4):
                    ps = f2ps.tile([128, 256], F32, tag="f2")
                    for k2 in range(NFP):
                        nc.tensor.matmul(
                            ps[:], hgT[:, k2, j * 128:(j + 1) * 128],
                            wf2[ccq][:, k2, :],
                            start=(k2 == 0), stop=(k2 == NFP - 1))
                    ya = opool.tile([128, 256], BF16, tag="ya")
                    nc.vector.tensor_tensor(
                        ya[:], ps[:], bfc2bc[:, ccq * 256:(ccq + 1) * 256],
                        op=ALU.add)
                    ost = opool.tile([128, 256], F32, tag="ost")
                    nc.vector.tensor_tensor(
                        ost[:], ya[:],
                        x2[j][:, ccq * 256:(ccq + 1) * 256], op=ALU.add)
                    nc.sync.dma_start(
                        out.ap()[j * 128:(j + 1) * 128,
                                 ccq * 256:(ccq + 1) * 256], ost[:])

    nc.compile()
    return nc


_NC_CACHE = None


def _get_program():
    global _NC_CACHE
    if _NC_CACHE is None:
        _NC_CACHE = build_program()
    return _NC_CACHE


def _prepare_in_maps(x, ln1_g, ln1_b, w_attn, b_attn, w_proj, b_proj,
                     ln2_g, ln2_b, w_fc, b_fc, w_fc2, b_fc2):
    bf = ml_dtypes.bfloat16
    x = np.asarray(x, np.float32)
    ln1_g = np.asarray(ln1_g, np.float32); ln1_b = np.asarray(ln1_b, np.float32)
    w_attn = np.asarray(w_attn, np.float32); b_attn = np.asarray(b_attn, np.float32)
    w_proj = np.asarray(w_proj, np.float32); b_proj = np.asarray(b_proj, np.float32)
    ln2_g = np.asarray(ln2_g, np.float32); ln2_b = np.asarray(ln2_b, np.float32)
    w_fc = np.asarray(w_fc, np.float32); b_fc = np.asarray(b_fc, np.float32)
    w_fc2 = np.asarray(w_fc2, np.float32); b_fc2 = np.asarray(b_fc2, np.float32)

    # Fold LayerNorm affine params into the following matmuls (exact).
    w_attn_f = ln1_g[:, None] * w_attn
    b_attn_f = b_attn + ln1_b @ w_attn
    w_fc_f = ln2_g[:, None] * w_fc
    b_fc_f = b_fc + ln2_b @ w_fc

    # single static staircase mask: mask[p, j] = 1 iff j >= p
    jj = np.arange(128)[None, :]
    pp = np.arange(128)[:, None]
    cmask = (jj >= pp).astype(bf)

    # weight pre-layouts for contiguous DMA runs
    # w_fc_f [C, FF]: [k(8),p(128)] x [m(32),c(128)] -> [p, m, k, c]
    wfc_p = np.ascontiguousarray(
        w_fc_f.reshape(NCP, 128, NFP, 128).transpose(1, 2, 0, 3)).astype(bf)
    # w_fc2 [FF, C]: [k2(32),p(128)] x [ccq(4),c(256)] -> [p, ccq, k2, c]
    wfc2_p = np.ascontiguousarray(
        w_fc2.reshape(NFP, 128, 4, 256).transpose(1, 2, 0, 3)).astype(bf)

    shared = {
        "cmask": cmask,
        "w_fc": wfc_p,
        "w_fc2": wfc2_p,
        "b_proj_bc": np.ascontiguousarray(
            np.broadcast_to(b_proj, (128, C))).astype(bf),
        "b_fc_col": np.ascontiguousarray(b_fc_f.reshape(32, 128).T),
        "b_fc2_bc": np.ascontiguousarray(
            np.broadcast_to(b_fc2, (128, C))).astype(bf),
    }

    in_maps = []
    for c in range(NCORES):
        bidx = c // 4
        g = c % 4
        fsl = slice(g * HGF, (g + 1) * HGF)
        w_q = w_attn_f[:, 0 * C:1 * C][:, fsl]
        w_k = w_attn_f[:, 1 * C:2 * C][:, fsl]
        w_v = w_attn_f[:, 2 * C:3 * C][:, fsl]
        b_q = b_attn_f[0 * C:1 * C][fsl]
        b_k = b_attn_f[1 * C:2 * C][fsl]
        b_v = b_attn_f[2 * C:3 * C][fsl]
        m = dict(shared)
        m["xb"] = np.ascontiguousarray(x[bidx]).astype(bf)
        # owned rows: 4 tiles of 128 rows at stride 512 (chunked RS layout)
        rows = x[bidx].reshape(4, 4, 128, C)[:, g]   # [qc, 128, C]
        m["xo"] = np.ascontiguousarray(rows.reshape(ROWS, C))
        wqkv = np.concatenate([w_q, w_k, w_v], axis=1)       # [C, 768]
        m["w_qkv"] = np.ascontiguousarray(
            wqkv.reshape(NCP, 128, 3 * HGF).transpose(1, 0, 2)).astype(bf)
        m["w_pr"] = np.ascontiguousarray(
            w_proj[fsl, :].reshape(2, 128, C)).astype(bf)
        m["b_qk_col"] = np.ascontiguousarray(
            np.concatenate([b_q, b_k]).reshape(4, 128).T)
        m["b_v_bc"] = np.ascontiguousarray(np.broadcast_to(b_v, (128, HGF)))
        in_maps.append(m)
    return in_maps


def _gather(res):
    y = np.empty((B, T, C), np.float32)
    for c in range(NCORES):
        bidx = c // 4
        g = c % 4
        o = res.results[c]["out"].reshape(4, 128, C)
        for qc in range(4):
            r0 = qc * 512 + g * 128
            y[bidx, r0:r0 + 128] = o[qc]
    return y


def kernel(**inputs):
    in_maps = _prepare_in_maps(**inputs)
    nc = _get_program()
    res = run_bass_kernel_spmd(nc, in_maps, core_ids=list(range(NCORES)))
    return _gather(res)


def run_traced(inputs, **kw):
    """Run with NTFF tracing; returns (output, BassKernelResults)."""
    in_maps = _prepare_in_maps(**inputs)
    nc = _get_program()
    res = run_bass_kernel_spmd(nc, in_maps, core_ids=list(range(NCORES)),
                               trace=True, **kw)
    return _gather(res), res


# revision 53
# speedup vs baseline: 1.0000x; 1.0000x over previous
"""Trainium2 Bass kernel for a dense transformer block (B=2, T=2048, C=1024, H=16).

Sharding (v4, pipelined tensor-parallel attention + chunked ReduceScatter):
  core c -> batch b = c//4, head-group g = c%4 (heads 4g..4g+3).
  After the CHUNKED ReduceScatter, core g owns token rows
  {qc*512 + 128*g + [0,128) : qc in 0..3} of its batch (strided, 4 tiles).

Emission is software-pipelined per query chunk qc so all five engines
overlap: LN1 stats/x-tile DMAs run two chunks ahead; LN-apply+transpose and
proj(qc-1) drain as background units inside C(qc).pt0's k-tile loop; qkv of
qc+1 drains inside C(qc).pt1; the ReduceScatter chunks fly under later
attention; E(j) residual+LN2 lags its RS by two chunks.  The MLP runs fc1
in token-column splits (256/128/128) so the last RS chunk + E(3) hide under
fc1, then a token-major fc2 (no output transposes).

Attention processes only valid (causal) score columns: diagonal-band k-tiles
compute/exp/mask just cols >= 128*band, with a single static 128x128
staircase mask.

Everything is bf16 except PSUM accumulation, LN statistics and the final
residual path (f32).  LayerNorm affines are folded into the following matmul
weights on the host (exact).  Weights are pre-laid-out on the host so every
DMA moves >=2KB contiguous runs per partition.
"""

from contextlib import ExitStack

import ml_dtypes
import numpy as np

import concourse.bass as bass
import concourse.tile as tile
import concourse.bacc as bacc
import concourse.mybir as mybir
from concourse.bass_utils import run_bass_kernel_spmd
from concourse.masks import make_identity

F32 = mybir.dt.float32
BF16 = mybir.dt.bfloat16
ALU = mybir.AluOpType
ACTF = mybir.ActivationFunctionType

B, T, C = 2, 2048, 1024
H, DH = 16, 64
FF = 4096
EPS = 1e-5
NCORES = 8
ROWS = 512            # token rows owned per core (MLP phase)
HG = 4                # heads per core
HGF = HG * DH         # 256 head-group features
NCP = C // 128        # 8 feature partition-tiles of C
NFP = FF // 128       # 32 feature partition-tiles of FF
NQC = T // 512        # 4 query chunks
VSTRIDE = DH + 1      # v stored with a ones column per head


def r(ap, pat, **kw):
    return ap.rearrange(pat, **kw)


def build_program():
    nc = bacc.Bacc("TRN2", target_bir_lowering=False, debug=False,
                   num_devices=NCORES)

    def din(name, shape, dtype=F32):
        return nc.dram_tensor(name, list(shape), dtype, kind="ExternalInput")

    xb = din("xb", (T, C), BF16)
    xo_d = din("xo", (ROWS, C))
    cmask = din("cmask", (128, 128), BF16)
    w_qkv = din("w_qkv", (128, NCP, 3 * HGF), BF16)   # [p, kt, q|k|v feats]
    w_pr = din("w_pr", (2, 128, C), BF16)             # proj rows, 2 p-tiles
    w_fc = din("w_fc", (128, NFP, NCP, 128), BF16)    # [p, m, kt, c]
    w_fc2 = din("w_fc2", (128, 4, NFP, 256), BF16)    # [p, ccq, k2, c]
    b_qk_col = din("b_qk_col", (128, 4))              # q0 q1 k0 k1 bias cols
    b_v_bc = din("b_v_bc", (128, HGF))
    b_proj_bc = din("b_proj_bc", (128, C), BF16)
    b_fc_col = din("b_fc_col", (128, 32))
    b_fc2_bc = din("b_fc2_bc", (128, C), BF16)
    out = nc.dram_tensor("out", [ROWS, C], F32, kind="ExternalOutput")

    with tile.TileContext(nc) as tc, ExitStack() as ctx:
        # ---- constants (whole-program lifetime) ----
        cpool = ctx.enter_context(tc.tile_pool(name="const", bufs=1))
        ident = cpool.tile([128, 128], F32, tag="ident")
        make_identity(nc, ident[:])
        identb = cpool.tile([128, 128], BF16, tag="identb")
        nc.vector.tensor_copy(identb[:], ident[:])
        bqk = cpool.tile([128, 4], F32, tag="bqk")
        bvbc = cpool.tile([128, HGF], F32, tag="bvbc")
        bprbc = cpool.tile([128, C], BF16, tag="bprbc")
        bfc = cpool.tile([128, 32], F32, tag="bfc")
        bfc2bc = cpool.tile([128, C], BF16, tag="bfc2bc")
        mtile = cpool.tile([128, 128], BF16, tag="mtile")
        epsc = cpool.tile([128, 1], F32, tag="epsc")
        nc.gpsimd.memset(epsc[:], EPS)
        # dummy Sqrt pulls its activation-table load to t=0 (otherwise it
        # gates the first LayerNorm ~6us into the run)
        warm = cpool.tile([128, 1], F32, tag="warm")
        nc.scalar.activation(warm[:], epsc[:], ACTF.Sqrt, bias=epsc[:],
                             scale=1.0)

        def load_consts():
            # deferred so these DMAs queue behind the critical first x tiles
            nc.sync.dma_start(bqk[:], b_qk_col.ap())
            nc.sync.dma_start(bvbc[:], b_v_bc.ap())
            nc.sync.dma_start(bprbc[:], b_proj_bc.ap())
            nc.sync.dma_start(bfc[:], b_fc_col.ap())
            nc.sync.dma_start(bfc2bc[:], b_fc2_bc.ap())
            nc.sync.dma_start(mtile[:], cmask.ap())

        def newton_rsqrt(spool, y, ve, iters):
            """y = rsqrt(ve) via Newton from seed y1 = 1.5 - 0.5*ve.
            All-DVE: keeps Sqrt (and its activation-table load) out of the
            ACT exp stream.  Valid for ve in ~(0, 2.5); LN variances here
            are ~1."""
            n = y.shape[-1]
            nc.vector.tensor_scalar(y, ve, -0.5, 1.5, op0=ALU.mult,
                                    op1=ALU.add)
            for _ in range(iters):
                t = spool.tile([128, n], F32, tag="nt")
                nc.vector.tensor_tensor(t[:], y, y, op=ALU.mult)
                nc.vector.tensor_tensor(t[:], t[:], ve, op=ALU.mult)
                nc.vector.tensor_scalar(t[:], t[:], -0.5, 1.5, op0=ALU.mult,
                                        op1=ALU.add)
                nc.vector.tensor_tensor(y, y, t[:], op=ALU.mult)

        def layernorm_apply(spool, xt, xn_out, iters=4):
            """xn_out = (xt - mean) * rsqrt(var + eps), rowwise over 1024."""
            st = spool.tile([128, 12], F32, tag="st")
            nc.vector.bn_stats(st[:, 0:6], xt[:, 0:512])
            nc.vector.bn_stats(st[:, 6:12], xt[:, 512:1024])
            ag = spool.tile([128, 2], F32, tag="ag")
            nc.vector.bn_aggr(ag[:], r(st, "p (c s) -> p c s", s=6))
            sd = spool.tile([128, 1], F32, tag="sd")
            nc.scalar.activation(sd[:], ag[:, 1:2], ACTF.Sqrt, bias=epsc[:],
                                 scale=1.0)
            rc = spool.tile([128, 1], F32, tag="rc")
            nc.vector.reciprocal(rc[:], sd[:])
            # xn = (x - mu) * r on DVE (bf16 in/out hits the 4x fast path)
            nc.vector.tensor_scalar(xn_out, xt, ag[:, 0:1], rc[:],
                                    op0=ALU.subtract, op1=ALU.mult)

        # DRAM bounce buffers for the chunked collective
        drpool = ctx.enter_context(tc.tile_pool(name="dram", bufs=1,
                                                space="DRAM"))
        pp_d = drpool.tile([T, C], BF16, tag="pp_d", name="pp_d")
        rs_d = drpool.tile([ROWS, C], BF16, tag="rs_d", name="rs_d")

        # ---- MLP-persistent state + shared transpose PSUM (outer scope,
        # so attention pools can close LIFO around them) ----
        mpers = ctx.enter_context(tc.tile_pool(name="mpers", bufs=1))
        x2 = [mpers.tile([128, C], F32, tag=f"x2{j}", name=f"x2{j}")
              for j in range(4)]
        xn2T = mpers.tile([128, NCP, ROWS], BF16, tag="xn2T", name="xn2T")
        hgT = mpers.tile([128, NFP, ROWS], BF16, tag="hgT", name="hgT")
        epool = ctx.enter_context(tc.tile_pool(name="phE", bufs=3))
        espool = ctx.enter_context(tc.tile_pool(name="phEs", bufs=4))
        wf0pool = ctx.enter_context(tc.tile_pool(name="wf0", bufs=1))
        wf0 = wf0pool.tile([128, 4, NCP, 128], BF16, tag="wf0", name="wf0")
        atps = ctx.enter_context(
            tc.tile_pool(name="atps", bufs=1, space="PSUM"))

        def phase_e(j, dma=None):
            dma = dma or nc.sync
            rs_sb = epool.tile([128, C], BF16, tag="rs")
            dma.dma_start(rs_sb[:], rs_d[j * 128:(j + 1) * 128, :])
            xot = epool.tile([128, C], F32, tag="xot")
            dma.dma_start(xot[:], xo_d.ap()[j * 128:(j + 1) * 128, :])
            xa = epool.tile([128, C], BF16, tag="xa")
            nc.vector.tensor_tensor(xa[:], rs_sb[:], bprbc[:], op=ALU.add)
            nc.vector.tensor_tensor(x2[j][:], xa[:], xot[:], op=ALU.add)
            xn2 = epool.tile([128, C], BF16, tag="xn2")
            layernorm_apply(espool, x2[j][:], xn2[:])
            for half in range(2):
                tp = atps.tile([128, 512], BF16, tag="tp",
                               name=f"etp_{j}_{half}")
                for pq in range(4):
                    pt = 4 * half + pq
                    nc.tensor.transpose(
                        tp[:, pq * 128:(pq + 1) * 128],
                        xn2[:, pt * 128:(pt + 1) * 128],
                        identb[:])
                nc.vector.tensor_copy(
                    xn2T[:, 4 * half:4 * half + 4, j * 128:(j + 1) * 128],
                    r(tp[:], "p (k c) -> p k c", c=128))

        # =================== attention super-phase =====================
        with ExitStack() as actx:
            kvp = actx.enter_context(tc.tile_pool(name="kv", bufs=1))
            kTb = [kvp.tile([128, T], BF16, tag=f"kT{i}", name=f"kT{i}")
                   for i in range(2)]
            vb = [kvp.tile([128, 2, HG * VSTRIDE], BF16, tag=f"v{i}",
                           name=f"v{i}") for i in range(T // 256)]
            wq = kvp.tile([128, NCP, 3 * HGF], BF16, tag="wq", name="wq")
            wp = [kvp.tile([128, C], BF16, tag=f"wp{i}", name=f"wp{i}")
                  for i in range(2)]
            xnTp = actx.enter_context(tc.tile_pool(name="xnT", bufs=1))
            qTp = actx.enter_context(tc.tile_pool(name="qT", bufs=1))
            yTp = actx.enter_context(tc.tile_pool(name="yT", bufs=1))
            apool = actx.enter_context(tc.tile_pool(name="phA", bufs=4))
            aspool = actx.enter_context(tc.tile_pool(name="phAs", bufs=6))
            smpool = actx.enter_context(tc.tile_pool(name="sm", bufs=3))
            atpool = actx.enter_context(tc.tile_pool(name="att", bufs=6))
            depool = actx.enter_context(tc.tile_pool(name="phDe", bufs=3))
            # PSUM: atps(outer) 0.5 + mm 2 + sc 3x1 + av 2 = 7.5 banks
            mmps = actx.enter_context(
                tc.tile_pool(name="mmps", bufs=2, space="PSUM"))
            scps = actx.enter_context(
                tc.tile_pool(name="scps", bufs=3, space="PSUM"))
            avps = actx.enter_context(
                tc.tile_pool(name="avps", bufs=1, space="PSUM"))

            def dma_x(qc):
                # prefetch the chunk's x tiles ~20us ahead so bn_stats never
                # holds the DVE queue waiting on a DMA
                tiles = []
                for tloc in range(4):
                    tt = 4 * qc + tloc
                    xt = apool.tile([128, C], BF16, tag="x", bufs=8,
                                    name=f"x_{tt}")
                    nc.sync.dma_start(xt[:],
                                      xb.ap()[tt * 128:(tt + 1) * 128, :])
                    tiles.append(xt)
                return tiles

            def stats_alloc():
                agq = aspool.tile([128, 4, 2], F32, tag="agq")
                rcq = aspool.tile([128, 4], F32, tag="rcq")
                sdq = aspool.tile([128, 4], F32, tag="sdq")
                return agq, rcq, sdq

            def stats_one(tloc, xt, agq, rcq, sdq, per_tile=False):
                st = aspool.tile([128, 12], F32, tag="st")
                nc.vector.bn_stats(st[:, 0:6], xt[:, 0:512])
                nc.vector.bn_stats(st[:, 6:12], xt[:, 512:1024])
                nc.vector.bn_aggr(agq[:, tloc, :],
                                  r(st, "p (c s) -> p c s", s=6))
                if per_tile:
                    nc.scalar.activation(sdq[:, tloc:tloc + 1],
                                         agq[:, tloc, 1:2], ACTF.Sqrt,
                                         bias=epsc[:], scale=1.0)
                    nc.vector.reciprocal(rcq[:, tloc:tloc + 1],
                                         sdq[:, tloc:tloc + 1])

            def phase_a_stats(qc, tiles):
                # LN stats (DVE) -- emitted ahead so the latency hides under
                # the previous exp stream
                agq, rcq, sdq = stats_alloc()
                for tloc in range(4):
                    stats_one(tloc, tiles[tloc], agq, rcq, sdq)
                nc.scalar.activation(sdq[:], agq[:, :, 1], ACTF.Sqrt,
                                     bias=epsc[:], scale=1.0)
                nc.vector.reciprocal(rcq[:], sdq[:])
                return tiles, agq, rcq

            def a_apply_units(qc, stats):
                """Per-tile LN-apply + transpose closures (bg units)."""
                tiles, agq, rcq = stats
                xnTq = xnTp.tile([128, NCP, 512], BF16, tag="xnT", bufs=2,
                                 name=f"xnT{qc}")

                def unit(tloc, xt):
                    xn = apool.tile([128, C], BF16, tag="xn")
                    nc.vector.tensor_scalar(xn[:], xt[:],
                                            agq[:, tloc, 0:1],
                                            rcq[:, tloc:tloc + 1],
                                            op0=ALU.subtract, op1=ALU.mult)
                    for half in range(2):
                        tp = atps.tile([128, 512], BF16, tag="tp",
                                       name=f"tp_{qc}_{tloc}_{half}")
                        for pq in range(4):
                            pt = 4 * half + pq
                            nc.tensor.transpose(
                                tp[:, pq * 128:(pq + 1) * 128],
                                xn[:, pt * 128:(pt + 1) * 128],
                                identb[:])
                        nc.vector.tensor_copy(
                            xnTq[:, 4 * half:4 * half + 4,
                                 tloc * 128:(tloc + 1) * 128],
                            r(tp[:], "p (k c) -> p k c", c=128))

                units = [lambda tloc=tloc, xt=xt: unit(tloc, xt)
                         for tloc, xt in enumerate(tiles)]
                return xnTq, units

            def b_units(qc, xnTq):
                """qkv matmul-group closures (bg units): 8 qk half-groups
                + 4 v groups."""
                qTb = [qTp.tile([128, 512], BF16, tag=f"qT{i}", bufs=2,
                                name=f"qT{i}_{qc}") for i in range(2)]

                def v_unit(tloc):
                    tt = 4 * qc + tloc
                    ps = mmps.tile([128, 512], F32, tag="mm",
                                   name=f"v_{tt}")
                    for kt in range(NCP):
                        nc.tensor.matmul(
                            ps[:, 0:HGF],
                            xnTq[:, kt, tloc * 128:(tloc + 1) * 128],
                            wq[:, kt, 2 * HGF:3 * HGF],
                            start=(kt == 0), stop=(kt == NCP - 1))
                    dst = r(vb[tt // 2][:, tt % 2, :], "p (h m) -> p h m",
                            m=VSTRIDE)[:, :, 0:DH]
                    nc.vector.tensor_tensor(
                        dst, r(ps[:, 0:HGF], "p (h m) -> p h m", m=DH),
                        r(bvbc[:], "p (h m) -> p h m", m=DH), op=ALU.add)

                units = [lambda tloc=tloc: v_unit(tloc) for tloc in range(4)]
                for m in range(4):
                    # m: 0,1 -> q head-pairs; 2,3 -> k head-pairs
                    holder = []

                    def qk_first(m=m, holder=holder):
                        ps = mmps.tile([128, 512], F32, tag="mm",
                                       name=f"qk_{qc}_{m}")
                        holder.append(ps)
                        for kt in range(4):
                            nc.tensor.matmul(
                                ps[:], wq[:, kt, m * 128:(m + 1) * 128],
                                xnTq[:, kt, :],
                                start=(kt == 0), stop=False)

                    def qk_second(m=m, holder=holder):
                        ps = holder[0]
                        for kt in range(4, NCP):
                            nc.tensor.matmul(
                                ps[:], wq[:, kt, m * 128:(m + 1) * 128],
                                xnTq[:, kt, :],
                                start=False, stop=(kt == NCP - 1))
                        if m < 2:
                            dst = qTb[m][:]
                        else:
                            dst = kTb[m - 2][:, qc * 512:(qc + 1) * 512]
                        nc.vector.tensor_scalar(dst, ps[:], bqk[:, m:m + 1],
                                                None, op0=ALU.add)

                    units += [qk_first, qk_second]
                return qTb, units

            def attn_head_pair(qc, pt, qTb, yTq, bg=None):
                """Both heads of pair pt, kt-pipelined: av lags sc by one
                k-tile so PE never waits on the exp.  One background unit
                (a closure emitting ~0.5-1us of boundary work) is drained
                per k-tile to fill PE slack under the ACT-paced stream."""
                nkt = 4 * (qc + 1)
                avs = [avps.tile([128, 512], F32, tag=f"av{sub}",
                                 name=f"av_{qc}_{pt}_{sub}")
                       for sub in range(2)]

                def emit_av(entries):
                    for sub, (kt, c0, et) in enumerate(entries):
                        h = 2 * pt + sub
                        nc.tensor.matmul(
                            avs[sub][0:VSTRIDE, c0:512],
                            vb[kt // 2][:, kt % 2,
                                        h * VSTRIDE:(h + 1) * VSTRIDE],
                            et[:, c0:512],
                            start=(kt == 0), stop=(kt == nkt - 1),
                            skip_group_check=True)

                prev = None
                for kt in range(nkt):
                    band = kt - 4 * qc
                    c0 = 128 * band if band > 0 else 0
                    cur = []
                    for sub in range(2):
                        hb = 64 * sub
                        sc = scps.tile([128, 512], F32, tag="sc")
                        nc.tensor.matmul(
                            sc[:, c0:512],
                            kTb[pt][hb:hb + 64, kt * 128:(kt + 1) * 128],
                            qTb[pt][hb:hb + 64, c0:512],
                            start=True, stop=True)
                        et = atpool.tile([128, 512], BF16, tag="e", bufs=6)
                        nc.scalar.activation(et[:, c0:512], sc[:, c0:512],
                                             ACTF.Exp, scale=0.125)
                        if band >= 0:
                            # staircase-mask the diagonal stripe (on Pool)
                            nc.gpsimd.tensor_tensor(
                                et[:, c0:c0 + 128], et[:, c0:c0 + 128],
                                mtile[:], op=ALU.mult)
                        cur.append((kt, c0, et))
                    if prev is not None:
                        emit_av(prev)
                    if bg:
                        bg.pop(0)()
                    prev = cur
                emit_av(prev)
                for sub in range(2):
                    hb = 64 * sub
                    rr = smpool.tile([1, 512], F32, tag="rr")
                    nc.vector.reciprocal(rr[:], avs[sub][DH:DH + 1, :])
                    bc = smpool.tile([64, 512], F32, tag="bc")
                    nc.gpsimd.partition_broadcast(bc[:], rr[:])
                    nc.vector.tensor_tensor(
                        yTq[pt][hb:hb + 64, :], avs[sub][0:DH, :], bc[:],
                        op=ALU.mult)

            def proj_unit(qc, tloc, yTq):
                tt = 4 * qc + tloc
                pe = depool.tile([128, C], BF16, tag="pe")
                pj = [mmps.tile([128, 512], F32, tag="mm",
                                name=f"pj_{tt}_{cc}") for cc in range(2)]
                for i in range(2):
                    for cc in range(2):
                        nc.tensor.matmul(
                            pj[cc][:],
                            yTq[i][:, tloc * 128:(tloc + 1) * 128],
                            wp[i][:, cc * 512:(cc + 1) * 512],
                            start=(i == 0), stop=(i == 1))
                nc.vector.tensor_copy(pe[:, 0:512], pj[0][:])
                nc.scalar.copy(pe[:, 512:1024], pj[1][:])
                nc.sync.dma_start(pp_d[tt * 128:(tt + 1) * 128, :], pe[:])

            def emit_rs(qc):
                nc.gpsimd.collective_compute(
                    "ReduceScatter", ALU.add,
                    replica_groups=[[0, 1, 2, 3], [4, 5, 6, 7]],
                    ins=[pp_d[qc * 512:(qc + 1) * 512, :]],
                    outs=[rs_d[qc * 128:(qc + 1) * 128, :]])

            def phase_d(qc, yTq):
                for tloc in range(4):
                    proj_unit(qc, tloc, yTq)
                emit_rs(qc)

            # ---- pipelined emission: stats of chunk qc+1 go ahead of
            # C(qc).pt0; LN-apply + proj(qc-1) drain as background units
            # inside pt0's kt loop; qkv(qc+1) units inside pt1's loop.
            # The exp stream never breaks and PE slack is backfilled. ----
            xts = {0: dma_x(0)}
            nc.sync.dma_start(wq[:], w_qkv.ap())
            for i in range(2):
                nc.sync.dma_start(wp[i][:], w_pr.ap()[i])
            xts[1] = dma_x(1)
            load_consts()
            for i in range(T // 256):
                nc.gpsimd.memset(
                    r(vb[i], "p s (h m) -> p s h m",
                      m=VSTRIDE)[:, :, :, DH:DH + 1], 1.0)
            # prologue: per-tile stats -> apply -> v interleave for min
            # latency (v of tile t only needs tile t transposed)
            agq0, rcq0, sdq0 = stats_alloc()
            xnTq, a_units = a_apply_units(0, (xts[0], agq0, rcq0))
            qTb, bu = b_units(0, xnTq)
            for tloc in range(4):
                stats_one(tloc, xts[0][tloc], agq0, rcq0, sdq0,
                          per_tile=True)
                a_units[tloc]()
                bu[tloc]()      # v unit for this tile
            for u in bu[4:]:
                u()
            yTq_prev = None
            for qc in range(NQC):
                yTq = [yTp.tile([128, 512], BF16, tag=f"yT{i}", bufs=2,
                                name=f"yT{i}_{qc}") for i in range(2)]
                if qc + 2 < NQC:
                    xts[qc + 2] = dma_x(qc + 2)
                bg0 = []
                if yTq_prev is not None:
                    bg0 += [lambda tloc=tloc, y=yTq_prev:
                            proj_unit(qc - 1, tloc, y) for tloc in range(4)]
                # E lags TWO chunks; as a bg unit its DVE chain lands in
                # pt0's slack instead of ahead of the normalize reciprocals
                if qc >= 2:
                    bg0.append(lambda j=qc - 2: phase_e(j))
                if qc + 1 < NQC:
                    stats_n = phase_a_stats(qc + 1, xts[qc + 1])
                    xnTq_n, a_un = a_apply_units(qc + 1, stats_n)
                    bg0 += a_un
                attn_head_pair(qc, 0, qTb, yTq, bg=bg0)
                for u in bg0:
                    u()
                bg0.clear()
                if qc + 1 < NQC:
                    qTb_n, bg1 = b_units(qc + 1, xnTq_n)
                else:
                    qTb_n, bg1 = None, []
                attn_head_pair(qc, 1, qTb, yTq, bg=bg1)
                for u in bg1:
                    u()
                bg1.clear()
                # RS(qc-1) emitted here so its SEQ-held wait on the pp
                # stores (long satisfied) never blocks Pool-queue masks
                if yTq_prev is not None:
                    emit_rs(qc - 1)
                if qc == 3:
                    phase_d(3, yTq)
                    phase_e(2)
                if qc == 2:
                    nc.sync.dma_start(wf0[:], w_fc.ap()[:, 0:4])
                qTb = qTb_n
                yTq_prev = yTq

        # =================== MLP super-phase ===========================
        with ExitStack() as mctx:
            fpool = mctx.enter_context(tc.tile_pool(name="phF", bufs=1))
            f2pool = mctx.enter_context(tc.tile_pool(name="phF2", bufs=2))
            opool = mctx.enter_context(tc.tile_pool(name="phO", bufs=3))
            fps = mctx.enter_context(
                tc.tile_pool(name="fps", bufs=2, space="PSUM"))
            f2ps = mctx.enter_context(
                tc.tile_pool(name="f2ps", bufs=2, space="PSUM"))

            # fc1 weights resident (mg=0 was prefetched during attention);
            # wf2 quarters prefetch
            wf = [wf0] + [fpool.tile([128, 4, NCP, 128], BF16, tag=f"wf{mg}",
                                     name=f"wf{mg}") for mg in range(1, 8)]
            for mg in range(1, 8):
                nc.sync.dma_start(wf[mg][:], w_fc.ap()[:, 4 * mg:4 * mg + 4])
            wf2 = [f2pool.tile([128, NFP, 256], BF16, tag="wf2",
                               name=f"wf2_{q}") for q in range(4)]
            nc.sync.dma_start(wf2[0][:], w_fc2.ap()[:, 0])
            nc.sync.dma_start(wf2[1][:], w_fc2.ap()[:, 1])

            def fc1_cols(lo, hi):
                for mg in range(8):
                    for mloc in range(4):
                        m = 4 * mg + mloc
                        ps = fps.tile([128, hi - lo], F32,
                                      tag=f"fc{hi - lo}", name=f"fc_{m}_{lo}")
                        for kt in range(NCP):
                            nc.tensor.matmul(
                                ps[:], wf[mg][:, mloc, kt, :],
                                xn2T[:, kt, lo:hi],
                                start=(kt == 0), stop=(kt == NCP - 1))
                        nc.scalar.activation(hgT[:, m, lo:hi], ps[:],
                                             ACTF.Gelu, bias=bfc[:, m:m + 1],
                                             scale=1.0)

            # chunks 0,1 first (E(0),E(1) long done); cols 256:384 follow
            # once E(2)'s chain lands; the last RS + E(3) hide underneath
            fc1_cols(0, 256)
            fc1_cols(256, 384)
            phase_e(3)
            fc1_cols(384, 512)

            # fc2 token-major in c-quarters, out stripes DMA'd as they finish
            for ccq in range(4):
                if ccq >= 2:
                    nc.sync.dma_start(wf2[ccq][:], w_fc2.ap()[:, ccq])
                for j in range(4):
                    ps = f2ps.tile([128, 256], F32, tag="f2")
                    for k2 in range(NFP):
                        nc.tensor.matmul(
                            ps[:], hgT[:, k2, j * 128:(j + 1) * 128],
                            wf2[ccq][:, k2, :],
                            start=(k2 == 0), stop=(k2 == NFP - 1))
                    ya = opool.tile([128, 256], BF16, tag="ya")
                    nc.vector.tensor_tensor(
                        ya[:], ps[:], bfc2bc[:, ccq * 256:(ccq + 1) * 256],
                        op=ALU.add)
                    ost = opool.tile([128, 256], F32, tag="ost")
                    nc.vector.tensor_tensor(
                        ost[:], ya[:],
                        x2[j][:, ccq * 256:(ccq + 1) * 256], op=ALU.add)
                    nc.sync.dma_start(
                        out.ap()[j * 128:(j + 1) * 128,
                                 ccq * 256:(ccq + 1) * 256], ost[:])

    nc.compile()
    return nc


_NC_CACHE = None


def _get_program():
    global _NC_CACHE
    if _NC_CACHE is None:
        _NC_CACHE = build_program()
    return _NC_CACHE


def _prepare_in_maps(x, ln1_g, ln1_b, w_attn, b_attn, w_proj, b_proj,
                     ln2_g, ln2_b, w_fc, b_fc, w_fc2, b_fc2):
    bf = ml_dtypes.bfloat16
    x = np.asarray(x, np.float32)
    ln1_g = np.asarray(ln1_g, np.float32); ln1_b = np.asarray(ln1_b, np.float32)
    w_attn = np.asarray(w_attn, np.float32); b_attn = np.asarray(b_attn, np.float32)
    w_proj = np.asarray(w_proj, np.float32); b_proj = np.asarray(b_proj, np.float32)
    ln2_g = np.asarray(ln2_g, np.float32); ln2_b = np.asarray(ln2_b, np.float32)
    w_fc = np.asarray(w_fc, np.float32); b_fc = np.asarray(b_fc, np.float32)
    w_fc2 = np.asarray(w_fc2, np.float32); b_fc2 = np.asarray(b_fc2, np.float32)

    # Fold LayerNorm affine params into the following matmuls (exact).
    w_attn_f = ln1_g[:, None] * w_attn
    b_attn_f = b_attn + ln1_b @ w_attn
    w_fc_f = ln2_g[:, None] * w_fc
    b_fc_f = b_fc + ln2_b @ w_fc

    # single static staircase mask: mask[p, j] = 1 iff j >= p
    jj = np.arange(128)[None, :]
    pp = np.arange(128)[:, None]
    cmask = (jj >= pp).astype(bf)

    # weight pre-layouts for contiguous DMA runs
    # w_fc_f [C, FF]: [k(8),p(128)] x [m(32),c(128)] -> [p, m, k, c]
    wfc_p = np.ascontiguousarray(
        w_fc_f.reshape(NCP, 128, NFP, 128).transpose(1, 2, 0, 3)).astype(bf)
    # w_fc2 [FF, C]: [k2(32),p(128)] x [ccq(4),c(256)] -> [p, ccq, k2, c]
    wfc2_p = np.ascontiguousarray(
        w_fc2.reshape(NFP, 128, 4, 256).transpose(1, 2, 0, 3)).astype(bf)

    shared = {
        "cmask": cmask,
        "w_fc": wfc_p,
        "w_fc2": wfc2_p,
        "b_proj_bc": np.ascontiguousarray(
            np.broadcast_to(b_proj, (128, C))).astype(bf),
        "b_fc_col": np.ascontiguousarray(b_fc_f.reshape(32, 128).T),
        "b_fc2_bc": np.ascontiguousarray(
            np.broadcast_to(b_fc2, (128, C))).astype(bf),
    }

    in_maps = []
    for c in range(NCORES):
        bidx = c // 4
        g = c % 4
        fsl = slice(g * HGF, (g + 1) * HGF)
        w_q = w_attn_f[:, 0 * C:1 * C][:, fsl]
        w_k = w_attn_f[:, 1 * C:2 * C][:, fsl]
        w_v = w_attn_f[:, 2 * C:3 * C][:, fsl]
        b_q = b_attn_f[0 * C:1 * C][fsl]
        b_k = b_attn_f[1 * C:2 * C][fsl]
        b_v = b_attn_f[2 * C:3 * C][fsl]
        m = dict(shared)
        m["xb"] = np.ascontiguousarray(x[bidx]).astype(bf)
        # owned rows: 4 tiles of 128 rows at stride 512 (chunked RS layout)
        rows = x[bidx].reshape(4, 4, 128, C)[:, g]   # [qc, 128, C]
        m["xo"] = np.ascontiguousarray(rows.reshape(ROWS, C))
        wqkv = np.concatenate([w_q, w_k, w_v], axis=1)       # [C, 768]
        m["w_qkv"] = np.ascontiguousarray(
            wqkv.reshape(NCP, 128, 3 * HGF).transpose(1, 0, 2)).astype(bf)
        m["w_pr"] = np.ascontiguousarray(
            w_proj[fsl, :].reshape(2, 128, C)).astype(bf)
        m["b_qk_col"] = np.ascontiguousarray(
            np.concatenate([b_q, b_k]).reshape(4, 128).T)
        m["b_v_bc"] = np.ascontiguousarray(np.broadcast_to(b_v, (128, HGF)))
        in_maps.append(m)
    return in_maps


def _gather(res):
    y = np.empty((B, T, C), np.float32)
    for c in range(NCORES):
        bidx = c // 4
        g = c % 4
        o = res.results[c]["out"].reshape(4, 128, C)
        for qc in range(4):
            r0 = qc * 512 + g * 128
            y[bidx, r0:r0 + 128] = o[qc]
    return y


def kernel(**inputs):
    in_maps = _prepare_in_maps(**inputs)
    nc = _get_program()
    res = run_bass_kernel_spmd(nc, in_maps, core_ids=list(range(NCORES)))
    return _gather(res)


def run_traced(inputs, **kw):
    """Run with NTFF tracing; returns (output, BassKernelResults)."""
    in_maps = _prepare_in_maps(**inputs)
    nc = _get_program()
    res = run_bass_kernel_spmd(nc, in_maps, core_ids=list(range(NCORES)),
                               trace=True, **kw)
    return _gather(res), res
